# revision 26
# baseline (speedup 1.0000x reference)
"""Trainium2 Bass kernel for the ConstraintLoss problem (8-core SPMD).

Contract: kernel(**inputs) takes the FULL unsharded inputs (numpy or jax
arrays, keyed as in setup_inputs()) and returns the full output — the
8-tuple of scalar losses stacked into a float32 array of shape (8,):
  [L_total, L_recon, L_rule, L_attn, L_attn_gat, L_attn_rule, L_reg,
   num_violations]

Sharding strategy (host side = structure prep + shard/unshard only; the
floating-point reduction math runs on the 8 NeuronCores):
  * Cars (180000) are sharded by ordinal range across the 8 cores
    (22528 rows/core, padded); score vectors follow the same row split.
  * The edge-wise segment-max over source-node segments is turned into a
    dense per-car reduction: the host bins each car's rule-edge payloads
    (payload = 1 - alpha, fp8; empty slot = 2.0) into a [rows, K=8]
    table; each core row-MIN-reduces its shard on the vector engine —
    the distributed segment-max from the sharding hint with the node
    space sharded so no cross-core combine is needed. Cars with more
    than K rule edges get the min of the extras folded into the last
    slot on the host (exact).
  * param0/param1 are shipped as fp8 (e4m3) and sharded by rows (512
    rows of each per core); each core computes its partial sum of
    squares, split between the ACT engine (Square+accumulate) and the
    DVE (scalar_tensor_tensor square+accumulate) so both engines finish
    together. fp8 quantization biases L_reg by ~1e-3 relative — far
    inside the 2e-2 gate.
  * Each core gathers its partial sums into a [128, 10] tile (single
    DVE writer, then one DMA — per-column accum_out targets shared
    across engines raced on HW); the host adds partitions and cores in
    float64 and applies the final scalar formula (the "all-reduce the
    scalar losses" step).

DMA traffic per core is ~4.43 MB (fp8 params + fp8 table + f32
scores), spread ~1.48 MB per DMA queue (sync/scalar HWDGE + gpsimd
SWDGE; each sustains ~120 GB/s here).
"""

import numpy as np
from contextlib import ExitStack

import ml_dtypes
import concourse.bacc as bacc
import concourse.mybir as mybir
import concourse.tile as tile
from concourse.bass_utils import run_bass_kernel_spmd

F32 = mybir.dt.float32
F16 = mybir.dt.float16
BF16 = mybir.dt.bfloat16
FP8 = mybir.dt.float8e4
ALU = mybir.AluOpType
ACTF = mybir.ActivationFunctionType

# Problem constants (hardcoded per the task contract).
N_CAR = 180000
N = 200000
NCORES = 8

G = 176                    # row groups per partition
RPC = 128 * G              # 22528 rows (car ordinals) per core
ROWS = RPC * NCORES        # 180224 padded rows
NPAD = ROWS - N_CAR        # 224 pad rows (all on core 7)
K = 8                      # rule-edge slots per car (min-fold handles overflow)
PF = 512 * 4096 // 128     # 16384 param elems per partition per core per param
PTOT = 2 * PF              # 32768: both params

# Param sum-of-squares split between ACT (Square, ~1.7 elem/ns/lane on
# fp8) and DVE (STT square+accum, ~1.9), AND across the three DMA
# queues (scalar/sync HWDGE, gpsimd SWDGE; ~120 GB/s each, ~1.53 MB
# per queue including tab+scores). Within each queue the last chunk is
# small (short tail after the final arrival) and queues end on
# different engines. Entries: ("pa"|"pd", width) = ACT/DVE param chunk;
# "sco"/"tab" are the score/table loads.
QUEUE_SEQS = {
    "scalar": [("sco", 0), ("pa", 4352), ("tab", 0), ("pd", 4928)],
    "sync": [("pd", 4352), ("pa", 4512), ("pa", 2880)],
    "gpsimd": [("pd", 4928), ("pa", 2240), ("pd", 4576)],
}
A_ELEMS = sum(w for s in QUEUE_SEQS.values() for k, w in s if k == "pa")
D_ELEMS = sum(w for s in QUEUE_SEQS.values() for k, w in s if k == "pd")
ACT_CH = sum(1 for s in QUEUE_SEQS.values() for k, _ in s if k == "pa")
DVE_CH = sum(1 for s in QUEUE_SEQS.values() for k, _ in s if k == "pd")
assert A_ELEMS + D_ELEMS == PTOT, (A_ELEMS, D_ELEMS)
# compute order per engine = expected arrival order (queue prefix MB)
def _arrival_order():
    acts, dves = [], []
    for q, seq in QUEUE_SEQS.items():
        pref = 0.0
        for k, w in seq:
            pref += {"sco": 3 * G * 2, "tab": G * K}.get(k, w) * 128
            if k == "pa":
                acts.append((pref, q, w))
            elif k == "pd":
                dves.append((pref, q, w))
    return sorted(acts), sorted(dves)
ACT_ORDER, DVE_ORDER = _arrival_order()


LAMBDA_RECON, LAMBDA_RULE, LAMBDA_ATTN, LAMBDA_REG = 1.0, 0.5, 0.3, 1e-4
W_ATTN_GAT, W_ATTN_RULE = 0.5, 0.5

_PAD_MS = 0.5              # pad rows: ms=rs=0.5 -> bce adds exactly ln(0.5)

_NC = None


def _build_nc(repeat=1, sections=("params", "smalls")):
    """Build + compile the per-core Bass program (SPMD; only the input
    shards differ). `repeat` unrolls the body for wall-clock timing;
    `sections` disables parts for perf bisection (output garbage when
    not all enabled)."""
    do_par = "params" in sections
    do_sm = "smalls" in sections
    nc = bacc.Bacc("TRN2", target_bir_lowering=False, debug=False,
                   enable_asserts=False, num_devices=NCORES)

    pq = nc.dram_tensor("pq", [128, PTOT], FP8, kind="ExternalInput")
    tab = nc.dram_tensor("tab", [128, G * K], FP8, kind="ExternalInput")
    # merged scores: cols [0,G)=model, [G,2G)=rule, [2G,3G)=beta
    sco = nc.dram_tensor("sco", [128, 3 * G], F16, kind="ExternalInput")
    outd = nc.dram_tensor("partials", [128, 10], F32, kind="ExternalOutput")

    with ExitStack() as ctx:
        tc = ctx.enter_context(tile.TileContext(nc))
        pp = ctx.enter_context(tc.tile_pool(name="pp", bufs=2))
        sc = ctx.enter_context(tc.tile_pool(name="sc", bufs=2))

        for _rep in range(repeat):
            # ---------------- DMAs (3 queues) ----------------
            engs = {"scalar": nc.scalar, "sync": nc.sync,
                    "gpsimd": nc.gpsimd}
            t_sco = None
            t_tab = None
            # param DRAM offsets keyed by (queue, seq position)
            pq_off = {}
            c0 = 0
            for q, seq in QUEUE_SEQS.items():
                for i, (k, w) in enumerate(seq):
                    if k in ("pa", "pd"):
                        pq_off[(q, i)] = c0
                        c0 += w
            assert c0 == PTOT
            # issue DMAs: per-queue sequences, interleaved round-robin
            tiles = {}
            maxlen = max(len(s) for s in QUEUE_SEQS.values())
            for i in range(maxlen):
                for q, seq in QUEUE_SEQS.items():
                    if i >= len(seq):
                        continue
                    k, w = seq[i]
                    if (k in ("sco", "tab") and not do_sm) or (
                            k in ("pa", "pd") and not do_par):
                        continue
                    if k == "sco":
                        t_sco = sc.tile([128, 3 * G], F16, tag="sco")
                        engs[q].dma_start(t_sco[:], sco.ap())
                    elif k == "tab":
                        t_tab = sc.tile([128, G * K], FP8, tag="tab")
                        engs[q].dma_start(t_tab[:], tab.ap())
                    else:
                        o = pq_off[(q, i)]
                        t = pp.tile([128, w], FP8, tag=f"{k}_{q}{i}")
                        engs[q].dma_start(t[:], pq.ap()[:, o:o + w])
                        tiles[(q, i)] = t
            t_ms = t_sco[:, 0:G] if do_sm else None
            t_rs = t_sco[:, G:2 * G] if do_sm else None
            t_bet = t_sco[:, 2 * G:3 * G] if do_sm else None
            # engine consumption order = arrival order; entries (tile, w)
            used = set()

            def _pick(order, kind):
                res = []
                for _, q, w in order:
                    i = next(i for i, (k2, w2) in enumerate(QUEUE_SEQS[q])
                             if k2 == kind and w2 == w
                             and (q, i) in tiles and (q, i) not in used)
                    used.add((q, i))
                    res.append((tiles[(q, i)], w))
                return res

            t_pa = _pick(ACT_ORDER, "pa") if do_par else []
            t_pd = _pick(DVE_ORDER, "pd") if do_par else []

            # ---------------- accumulators ----------------
            # One tile per engine: per-column accum_out targets shared
            # across engines + direct DMA of accum targets raced on HW
            # (nondeterministic scnt/sgat); the proven-safe pattern is
    # single-writer accum tiles, a DVE gather, then the DMA.
            out_a = sc.tile([128, 2], F32, tag="out_a")     # ACT: ln2, sqd
            out_d = sc.tile([128, 6], F32, tag="out_d")     # DVE scalars
            acc_a = sc.tile([128, ACT_CH], F32, tag="acc_a")
            acc_d = sc.tile([128, DVE_CH], F32, tag="acc_d")

            # ---------------- ACT program ----------------
            if do_sm:
              ln1 = sc.tile([128, G], F16, tag="ln1")
              nc.scalar.activation(ln1[:], t_ms, ACTF.Ln)
              ln2 = sc.tile([128, G], F16, tag="ln2")
              nc.scalar.activation(ln2[:], t_ms, ACTF.Ln, scale=-1.0,
                                   bias=1.0, accum_out=out_a[:, 0:1])
              bsq = sc.tile([128, G], F32, tag="bsq")
              nc.scalar.activation(bsq[:], t_bet, ACTF.Square, scale=-1.0,
                                   bias=1.0)
            if do_par:
              aw_max = max(w for _, w in t_pa)
              a_scr = sc.tile([128, aw_max], BF16, tag="a_scr")
              nc.scalar.activation(a_scr[:, :t_pa[0][1]], t_pa[0][0][:],
                                   ACTF.Square, accum_out=acc_a[:, 0:1])

            # ---------------- DVE program (early smalls) ----------------
            if do_sm:
              rowmin = sc.tile([128, G], F32, tag="rowmin")
              nc.vector.tensor_reduce(
                rowmin[:], t_tab[:].rearrange("p (g k) -> p g k", k=K),
                mybir.AxisListType.X, ALU.min)
            if do_sm:
              viol = sc.tile([128, G], F32, tag="viol")
              nc.vector.tensor_scalar(viol[:], t_rs, 0.5, 0.0, ALU.is_gt,
                                      ALU.add, accum_out=out_d[:, 2:3])
              scr_g = sc.tile([128, G], F16, tag="scr_g")
              nc.vector.scalar_tensor_tensor(scr_g[:], t_rs, 1.0, ln1[:],
                                             ALU.mult, ALU.mult,
                                             accum_out=out_d[:, 0:1])
              scr_g2 = sc.tile([128, G], F16, tag="scr_g2")
              nc.vector.scalar_tensor_tensor(scr_g2[:], t_rs, 1.0, ln2[:],
                                             ALU.mult, ALU.mult,
                                             accum_out=out_d[:, 1:2])
              diff = sc.tile([128, G], F16, tag="diff")
              nc.vector.tensor_tensor(diff[:], t_ms, t_rs, ALU.subtract)
              scr_g3 = sc.tile([128, G], F32, tag="scr_g3")
              nc.vector.scalar_tensor_tensor(scr_g3[:], viol[:], 1.0, bsq[:],
                                             ALU.mult, ALU.mult,
                                             accum_out=out_d[:, 3:4])

              # -------------- ACT follow-ups --------------
              dd2 = sc.tile([128, G], F32, tag="dd2")
              nc.scalar.activation(dd2[:], rowmin[:], ACTF.Square)
              sq_d = sc.tile([128, G], BF16, tag="sq_d")
              nc.scalar.activation(sq_d[:], diff[:], ACTF.Square,
                                   accum_out=out_a[:, 1:2])
            for c in range(1, ACT_CH):
                if not do_par:
                    break
                t, w = t_pa[c]
                a_scr = sc.tile([128, aw_max], BF16, tag="a_scr")
                nc.scalar.activation(a_scr[:, :w], t[:], ACTF.Square,
                                     accum_out=acc_a[:, c:c + 1])

            # ---------------- DVE follow-ups ----------------
            # param chunks in arrival order; the valid/gx pair needs
            # ACT's dd2, which is only ready mid-body — doing them last
            # keeps DVE from stalling.
            for c in range(DVE_CH):
                if not do_par:
                    break
                t, w = t_pd[c]
                dw_max = max(w2 for _, w2 in t_pd)
                d_scr = sc.tile([128, dw_max], BF16, tag="d_scr")
                nc.vector.scalar_tensor_tensor(
                    d_scr[:, :w], t[:], 1.0, t[:], ALU.mult, ALU.mult,
                    accum_out=acc_d[:, c:c + 1])
            if do_sm:
              # valid = (rowmin <= 1, i.e. car has a rule edge) * viol
              valid = sc.tile([128, G], F32, tag="valid")
              nc.vector.scalar_tensor_tensor(valid[:], rowmin[:], 1.0,
                                             viol[:], ALU.is_le, ALU.mult,
                                             accum_out=out_d[:, 4:5])
              scr_g4 = sc.tile([128, G], F32, tag="scr_g4")
              nc.vector.scalar_tensor_tensor(scr_g4[:], valid[:], 1.0,
                                             dd2[:], ALU.mult, ALU.mult,
                                             accum_out=out_d[:, 5:6])
            if not do_sm:
                nc.vector.memset(out_a[:], 0.0)
                nc.vector.memset(out_d[:], 0.0)
            if not do_par:
                nc.vector.memset(acc_a[:], 0.0)
                nc.vector.memset(acc_d[:], 0.0)

            # gather on DVE (single writer), then one DMA. Column map:
            # 0:c1 1:c2 2:nv 3:sar 4:scnt 5:sgat 6:ln2 7:srule 8:sp_a 9:sp_d
            parts = sc.tile([128, 10], F32, tag="parts")
            nc.vector.tensor_copy(parts[:, 0:6], out_d[:])
            nc.vector.tensor_copy(parts[:, 6:8], out_a[:])
            nc.vector.tensor_reduce(parts[:, 8:9], acc_a[:],
                                    mybir.AxisListType.X, ALU.add)
            nc.vector.tensor_reduce(parts[:, 9:10], acc_d[:],
                                    mybir.AxisListType.X, ALU.add)
            nc.sync.dma_start(outd.ap(), parts[:])

    nc.compile()
    return nc


def _get_nc():
    global _NC
    if _NC is None:
        _NC = _build_nc()
    return _NC


def prep_in_maps(inputs):
    """Host-side structure prep + sharding. Returns per-core input dicts."""
    ms = np.asarray(inputs["model_scores"], np.float32)
    rsv = np.asarray(inputs["rule_scores"], np.float32)
    alpha = np.asarray(inputs["alpha_gat"], np.float32)
    beta = np.asarray(inputs["beta_rule"], np.float32)
    ei = np.asarray(inputs["edge_index"])
    et = np.asarray(inputs["entity_types"])
    p0 = np.ascontiguousarray(np.asarray(inputs["param0"], np.float32))
    p1 = np.ascontiguousarray(np.asarray(inputs["param1"], np.float32))

    src = ei[0].astype(np.int64, copy=False)
    dst = ei[1].astype(np.int64, copy=False)

    # rule edges: dst is a light (1) or stop line (2)
    rule_node = (et == 1) | (et == 2)
    sel = rule_node[dst]
    src_r = src[sel]
    a_r = alpha[sel]

    # group rule-edge payloads (1 - alpha) by source node (CSR-style)
    order = np.argsort(src_r, kind="stable")
    ssrc = src_r[order]
    pay = np.float32(1.0) - a_r[order]
    counts = np.bincount(ssrc, minlength=N)
    starts = np.zeros_like(counts)
    starts[1:] = np.cumsum(counts[:-1])

    # car ordinal -> node id (reference: nonzero(et==0, size=N_CAR), fill 0)
    car_ids = np.nonzero(et == 0)[0]
    if car_ids.size >= N_CAR:
        car_ids = car_ids[:N_CAR]
    else:
        car_ids = np.concatenate(
            [car_ids, np.zeros(N_CAR - car_ids.size, car_ids.dtype)])

    # [ROWS, K] fp16 table of payloads; empty slots = 2.0 (> any payload)
    cnt_full = counts[car_ids]
    cnt_ord = np.minimum(cnt_full, K)
    tot = int(cnt_ord.sum())
    row_idx = np.repeat(np.arange(N_CAR, dtype=np.int64), cnt_ord)
    cum = np.cumsum(cnt_ord) - cnt_ord
    within = np.arange(tot, dtype=np.int64) - np.repeat(cum, cnt_ord)
    srcpos = np.repeat(starts[car_ids], cnt_ord) + within
    ptab = np.full(ROWS * K, 2.0, np.float16)
    ptab[row_idx * K + within] = pay[srcpos]
    # overflow fold (degree > K): min of the extras into the last slot
    for i in np.nonzero(cnt_full > K)[0]:
        node = car_ids[i]
        extra = pay[starts[node] + K:starts[node] + cnt_full[i]]
        ptab[i * K + K - 1] = min(ptab[i * K + K - 1],
                                  np.float16(extra.min()))
    ptab = ptab.reshape(ROWS, K).astype(ml_dtypes.float8_e4m3)

    # padded score rows
    def pad(v, fill):
        o = np.full(ROWS, fill, np.float32)
        o[:N_CAR] = v
        return o

    # fp16 shipping: ms near 1 would round to exactly 1.0 and make
    # Ln(1-ms) = -inf; clamp to the largest fp16 strictly below 1.
    ms_p = pad(np.minimum(ms, np.float32(1.0 - 2.0 ** -11)), _PAD_MS)
    rs_p = pad(rsv, _PAD_MS)   # pad: bce term exactly ln(0.5); never a viol
    bet_p = pad(beta, 1.0)

    # both params, fp8: [1024, 4096] rows per core -> [128, 32768]
    pq = np.concatenate([p0.reshape(NCORES, 512 * 4096 // PF, PF),
                         p1.reshape(NCORES, 512 * 4096 // PF, PF)],
                        axis=1).astype(ml_dtypes.float8_e4m3)
    # shape now [NCORES, 256, 16384]: per core [128(+128), 16384] halves
    # -> rearrange to [128, 32768] with p0 in cols :16384, p1 in 16384:
    pq = pq.reshape(NCORES, 2, 128, PF).transpose(0, 2, 1, 3).reshape(
        NCORES, 128, PTOT)

    in_maps = []
    for c in range(NCORES):
        r0, r1 = c * RPC, (c + 1) * RPC
        scov = np.concatenate([ms_p[r0:r1].reshape(128, G),
                               rs_p[r0:r1].reshape(128, G),
                               bet_p[r0:r1].reshape(128, G)],
                              axis=1).astype(np.float16)
        in_maps.append({
            "pq": np.ascontiguousarray(pq[c]),
            "tab": np.ascontiguousarray(ptab[r0:r1]).reshape(128, G * K),
            "sco": np.ascontiguousarray(scov),
        })
    return in_maps


def combine_partials(results):
    """Host unshard: sum partial tiles over partitions+cores (f64), apply
    the final scalar formula."""
    s = np.zeros(10, np.float64)
    for r in results:
        s += np.asarray(r["partials"], np.float64).reshape(128, 10).sum(axis=0)
    sc1, sc2, nv, sar, scnt, sgat, sln2, srule = s[:8]
    sp = s[8] + s[9]
    bce_sum = sc1 + sln2 - sc2
    bce_sum -= NPAD * np.log(0.5)  # remove the constant pad-row contribution

    L_recon = -bce_sum / N_CAR
    L_rule = srule / N_CAR
    any_viol = nv > 0
    L_attn_gat = (sgat / max(scnt, 1.0)) if (any_viol and scnt > 0) else 0.0
    L_attn_rule = (sar / max(nv, 1.0)) if any_viol else 0.0
    L_attn = W_ATTN_GAT * L_attn_gat + W_ATTN_RULE * L_attn_rule
    L_reg = sp
    L_total = (LAMBDA_RECON * L_recon + LAMBDA_RULE * L_rule
               + LAMBDA_ATTN * L_attn + LAMBDA_REG * L_reg)
    return np.array([L_total, L_recon, L_rule, L_attn, L_attn_gat,
                     L_attn_rule, L_reg, nv], np.float32)


def kernel(**inputs):
    nc = _get_nc()
    in_maps = prep_in_maps(inputs)
    res = run_bass_kernel_spmd(nc, in_maps, list(range(NCORES)))
    return combine_partials(res.results)
